# revision 31
# baseline (speedup 1.0000x reference)
"""Varlen causal GQA attention on 8 TRN2 NeuronCores.

Sharding: tensor-parallel over heads. Core c gets KV head c and its 4
query heads (GQA group), so every core runs an identical program on its
own head-slice of q/k/v and produces its own head-slice of the output.
No cross-core communication.

Per core, per (sequence, 256-row query block):
  - Q^T for the 4 heads via PE transposes (f32) + cast-to-bf16 copies
    into one [d, head, block_col] tile; K^T likewise, per sequence.
  - For each 128-row KV tile j: S^T [kv, head, q_col] = two head-pair
    matmuls (bf16 in, f32 PSUM out), column-sliced to the exact causal
    extent; then ONE exp over all 4 heads on ScalarE -> bf16 A^T in
    SBUF (no max subtraction: logits are O(1) so exp is safe), with the
    causal triangle of the diagonal tile zeroed by a GpSimd
    affine_select.
  - O [q, head, d | rowsum] accumulated in PSUM over j via
    matmul(lhsT=A^T_j, rhs=[V_j | ones]); the ones column yields the
    softmax denominator in the same matmul.
  - normalize with reciprocal + a broadcast tensor-tensor multiply and
    DMA out (stores issued on the GpSimd SWDGE queue to offload SP).

The image's walrus encodes at most 1 sem-wait per instruction, so a
post-pass hoists excess Tile-generated waits onto EventSemaphore
carriers (see _split_excess_waits).
"""

import os
import sys

import numpy as np

for _p in ("/opt/trn_rl_repo", "/root/.axon_site/_ro/trn_rl_repo"):
    if os.path.isdir(_p) and _p not in sys.path:
        sys.path.insert(0, _p)

NUM_HEADS = 32
NUM_KV_HEADS = 8
HEAD_DIM = 128
SCALE = 0.08838834764831845  # head_dim ** -0.5
N_CORES = 8
HPC = NUM_HEADS // N_CORES  # q heads per core = 4
DQ = HPC * HEAD_DIM  # 512

_BUILD_CACHE = {}
LAST_RESULT = None

# The walrus in this image only encodes 1 sem-wait per instruction; Tile's
# kernel-tail drain accumulates one wait per live semaphore. Split it into a
# chain of drains, each carrying at most one wait.
_MAX_WAITS = 1
_drain_patched = False


def _patch_tile_drain():
    global _drain_patched
    if _drain_patched:
        return
    import concourse.tile as tile
    from concourse import mybir
    from concourse.vector_clock import ScopedClock

    def _drain_and_barrier(self, tick_clock, wait_clock):
        nc = self.nc
        drain_inst = nc.sync.drain()
        wait_clock.add_sem_waits(
            drain_inst.ins, ScopedClock({None: tick_clock.global_clock})
        )
        si = drain_inst.ins.sync_info
        waits = list(si.on_wait) if si is not None and si.on_wait else []
        if len(waits) > _MAX_WAITS:
            drain_inst.ins.sync_info = mybir.SyncInfo(
                on_wait=waits[:_MAX_WAITS],
                on_update=list(si.on_update) if si.on_update else [],
            )
            for i in range(_MAX_WAITS, len(waits), _MAX_WAITS):
                extra = nc.sync.drain()
                extra.ins.sync_info = mybir.SyncInfo(
                    on_wait=waits[i : i + _MAX_WAITS], on_update=[]
                )
        nc.all_engine_barrier()
        assert self.sems is not None
        popped = nc._tile_sem_poison_stack.pop()
        assert popped is self._sem_poison
        nc.clear_and_free_semaphores(list(self.sems.allocated().values()))
        nc.all_engine_barrier()

    tile.TileContext._drain_and_barrier = _drain_and_barrier
    _drain_patched = True


def _split_excess_waits(nc):
    """The walrus in this image encodes at most 1 sem-wait per instruction
    (2 for Drain). Tile emits up to ~3. Hoist excess waits onto standalone
    EventSemaphore carriers on the same engine, inserted just before the
    over-limit instruction (same-engine program order preserves semantics).
    """
    from concourse import mybir

    n = 0
    for bb in nc.main_func.blocks:
        out = []
        for ins in bb.instructions:
            si = getattr(ins, "sync_info", None)
            waits = list(si.on_wait) if si is not None and si.on_wait else []
            limit = 1
            if len(waits) > limit:
                for w in waits[:-limit]:
                    n += 1
                    out.append(
                        mybir.InstEventSemaphore(
                            name=f"WSPLIT-{n}",
                            engine=ins.engine,
                            sync_info=mybir.SyncInfo(on_wait=[w], on_update=[]),
                            ins=[],
                            outs=[],
                        )
                    )
                ins.sync_info = mybir.SyncInfo(
                    on_wait=waits[-limit:],
                    on_update=list(si.on_update) if si.on_update else [],
                )
            out.append(ins)
        bb.instructions[:] = out
    return n


def _build(lens):
    import concourse.bass as bass
    import concourse.tile as tile
    from concourse import mybir
    from concourse.bass import ds, ts
    from concourse.masks import make_identity

    _patch_tile_drain()

    f32 = mybir.dt.float32
    bf16 = mybir.dt.bfloat16
    T = int(sum(lens))

    nc = bass.Bass()
    q_d = nc.declare_dram_parameter("q", [T, DQ], f32, isOutput=False)
    k_d = nc.declare_dram_parameter("k", [T, HEAD_DIM], f32, isOutput=False)
    v_d = nc.declare_dram_parameter("v", [T, HEAD_DIM], f32, isOutput=False)
    o_d = nc.declare_dram_parameter("out", [T, DQ], f32, isOutput=True)

    with tile.TileContext(nc) as tc:
        with (
            tc.tile_pool(name="consts", bufs=1) as consts,
            tc.tile_pool(name="kvseq", bufs=4) as kvseq,
            tc.tile_pool(name="work", bufs=6) as work,
            tc.tile_pool(name="qtp", bufs=12) as qtp,
            tc.tile_pool(name="aexp", bufs=22) as aexp,
            tc.tile_pool(name="ps_t", bufs=2, space="PSUM") as ps_t,
            tc.tile_pool(name="ps_s", bufs=2, space="PSUM") as ps_s,
            tc.tile_pool(name="ps_o", bufs=2, space="PSUM") as ps_o,
        ):
            ident = consts.tile([128, 128], f32)
            make_identity(nc, ident)
            # tri[p, f] = 1 if f >= p else 0  (keep q_pos >= kv_pos on the
            # diagonal tile of S^T, where partitions=kv and free=q)
            tri = consts.tile([128, 128], bf16)
            nc.gpsimd.memset(tri, 1.0)
            nc.gpsimd.affine_select(
                out=tri,
                in_=tri,
                compare_op=mybir.AluOpType.is_ge,
                fill=0.0,
                base=0,
                pattern=[[1, 128]],
                channel_multiplier=-1,
            )

            # Warm the PE HAM clock gate during the initial DMA loads:
            # ~3.5us of dummy matmuls lift PE from 1.2 to 2.4 GHz before
            # real work arrives. One accumulation group so DCE keeps them;
            # one throwaway read at the end.
            warm_ps = ps_t.tile([128, 128], f32, tag="tp")
            NWARM = 56
            for w in range(NWARM):
                nc.tensor.matmul(
                    warm_ps[:],
                    tri[:],
                    tri[:],
                    start=(w == 0),
                    stop=(w == NWARM - 1),
                )
            warm_sink = consts.tile([128, 1], f32)
            nc.vector.tensor_copy(warm_sink[:], warm_ps[:, 0:1])

            # Process a short sequence first (fast pipeline fill at the
            # head) and the shortest last (minimal drain tail); the long
            # sequences run mid-kernel where the pipeline is saturated.
            offs = []
            _o = 0
            for L in lens:
                offs.append(_o)
                _o += int(L)
            order = sorted(
                range(len(lens)), key=lambda i: (int(lens[i]),)
            )
            seq_order = (
                [order[1]]
                + [i for i in range(len(lens)) if i not in (order[0], order[1])]
                + [order[0]]
            )
            for _si in seq_order:
                L = int(lens[_si])
                off = offs[_si]
                nt = (L + 127) // 128
                nfull = L // 128
                rrem = L - nfull * 128

                # ---- K: load natural layout, PE-transpose to K^T bf16 ----
                k_nat = kvseq.tile([128, 8, 128], f32, tag="k_nat")
                if nfull:
                    nc.sync.dma_start(
                        out=k_nat[:, 0:nfull, :],
                        in_=k_d[off : off + nfull * 128, :].rearrange(
                            "(t p) d -> p t d", p=128
                        ),
                    )
                if rrem:
                    nc.sync.dma_start(
                        out=k_nat[:rrem, nfull, :],
                        in_=k_d[off + nfull * 128 : off + L, :],
                    )
                kt = kvseq.tile([128, 8 * 128], bf16, tag="kt")
                for j in range(nt):
                    jr = 128 if j < nfull else rrem
                    tp = ps_t.tile([128, 128], f32, tag="tp")
                    nc.tensor.transpose(
                        tp[:, :jr], k_nat[:jr, j, :], ident[:jr, :jr]
                    )
                    nc.any.tensor_copy(kt[:, ds(j * 128, jr)], tp[:, :jr])

                # ---- V: load natural layout, cast to bf16, append ones col ----
                v_nat = kvseq.tile([128, 8, 128], f32, tag="v_nat")
                if nfull:
                    nc.sync.dma_start(
                        out=v_nat[:, 0:nfull, :],
                        in_=v_d[off : off + nfull * 128, :].rearrange(
                            "(t p) d -> p t d", p=128
                        ),
                    )
                if rrem:
                    nc.sync.dma_start(
                        out=v_nat[:rrem, nfull, :],
                        in_=v_d[off + nfull * 128 : off + L, :],
                    )
                v_sb = kvseq.tile([128, 8, 136], bf16, tag="v_sb")
                if nfull:
                    nc.vector.tensor_copy(
                        v_sb[:, 0:nfull, 0:128], v_nat[:, 0:nfull, :]
                    )
                if rrem:
                    nc.vector.tensor_copy(
                        v_sb[:rrem, nfull, 0:128], v_nat[:rrem, nfull, :]
                    )
                nc.vector.memset(v_sb[:, 0:nt, 128:129], 1.0)

                # ---- main attention loops: blocks of 2 query tiles ----
                nblocks = (nt + 1) // 2
                for b in range(nblocks):
                    t_tiles = [t for t in (0, 1) if b * 2 + t < nt]
                    irs = [
                        128 if b * 2 + t < nfull else rrem for t in t_tiles
                    ]
                    bcols = sum(irs)
                    jmax = b * 2 + t_tiles[-1]

                    # load the block's q tiles [rows, 512] f32 in one DMA
                    brow0 = off + b * 256
                    nqfull = sum(1 for ir in irs if ir == 128)
                    q_nat = work.tile([128, 2, DQ], f32, tag="q_nat")
                    if nqfull:
                        nc.sync.dma_start(
                            out=q_nat[:, 0:nqfull, :],
                            in_=q_d[brow0 : brow0 + nqfull * 128, :].rearrange(
                                "(t p) d -> p t d", p=128
                            ),
                        )
                    if nqfull < len(irs):
                        rq = irs[nqfull]
                        nc.sync.dma_start(
                            out=q_nat[:rq, nqfull, :],
                            in_=q_d[
                                brow0 + nqfull * 128 : brow0 + nqfull * 128 + rq, :
                            ],
                        )
                    q_nats = [q_nat[:, t, :] for t in t_tiles]

                    # Q^T for all 4 heads: [d, head, block_col] bf16
                    qt_all = qtp.tile([128, HPC, 256], bf16, tag="qt")
                    for hp in range(2):  # head pairs
                        tp = ps_t.tile([128, 512], f32, tag="tp")
                        for hh in range(2):
                            h = hp * 2 + hh
                            for t, ir in zip(t_tiles, irs):
                                nc.tensor.transpose(
                                    tp[:, ds(hh * 256 + t * 128, ir)],
                                    q_nats[t][:ir, ts(h, 128)],
                                    ident[:ir, :ir],
                                )
                        nc.any.tensor_copy(
                            qt_all[:, hp * 2 : hp * 2 + 2, :bcols],
                            tp[:, 0:512].rearrange(
                                "p (h c) -> p h c", c=256
                            )[:, :, :bcols],
                        )

                    # scores + exp for every kv tile against the whole block
                    a_sbs = []
                    for j in range(jmax + 1):
                        jr = 128 if j < nfull else rrem
                        col0 = max(0, (j - b * 2) * 128)
                        s_big = ps_s.tile([128, HPC, 256], f32, tag="s_big")
                        for hp in range(2):
                            nc.tensor.matmul(
                                s_big[:jr, hp * 2 : hp * 2 + 2, col0:bcols],
                                kt[:, ds(j * 128, jr)],
                                qt_all[:, hp * 2 : hp * 2 + 2, col0:bcols],
                            )
                        a_sb = aexp.tile([128, HPC, 256], bf16, tag="a_sb")
                        nc.scalar.activation(
                            out=a_sb[:jr, :, col0:bcols],
                            in_=s_big[:jr, :, col0:bcols],
                            func=mybir.ActivationFunctionType.Exp,
                            scale=SCALE,
                        )
                        if j >= b * 2:
                            # diagonal tile: zero a[j,c] where c < j (causal)
                            nc.gpsimd.affine_select(
                                out=a_sb[:jr, :, col0 : col0 + jr],
                                in_=a_sb[:jr, :, col0 : col0 + jr],
                                compare_op=mybir.AluOpType.is_ge,
                                fill=0.0,
                                base=0,
                                pattern=[[0, HPC], [1, jr]],
                                channel_multiplier=-1,
                            )
                        a_sbs.append(a_sb)

                    # O accumulation, normalize, store per query tile
                    for t, ir in zip(t_tiles, irs):
                        i = b * 2 + t
                        row0 = off + i * 128
                        out_sb = work.tile([128, DQ], f32, tag="out_sb")
                        for hp in range(2):
                            o_ps = ps_o.tile([128, 2, 129], f32, tag="o_ps")
                            for hh in range(2):
                                h = hp * 2 + hh
                                for j in range(i + 1):
                                    jr = 128 if j < nfull else rrem
                                    nc.tensor.matmul(
                                        o_ps[:ir, hh, :],
                                        a_sbs[j][
                                            :jr, h, t * 128 : t * 128 + ir
                                        ],
                                        v_sb[:jr, j, 0:129],
                                        start=(j == 0),
                                        stop=(j == i),
                                    )
                            recip = work.tile([128, 2], f32, tag="recip")
                            nc.vector.reciprocal(
                                recip[:ir, :], o_ps[:ir, :, 128]
                            )
                            recip_bc = bass.AP(
                                tensor=recip.tensor,
                                offset=recip.offset,
                                ap=[recip.ap[0][:], [recip.ap[1][0], 2], [0, 128]],
                            )[:ir]
                            nc.vector.tensor_mul(
                                out_sb[:ir, ds(hp * 256, 256)].rearrange(
                                    "p (h c) -> p h c", c=128
                                ),
                                o_ps[:ir, :, 0:128],
                                recip_bc,
                            )
                        nc.gpsimd.dma_start(
                            out=o_d[row0 : row0 + ir, :], in_=out_sb[:ir, :]
                        )
    _split_excess_waits(nc)
    return nc


def _get_program(lens):
    key = tuple(int(x) for x in lens)
    if key not in _BUILD_CACHE:
        _BUILD_CACHE[key] = _build(key)
    return _BUILD_CACHE[key]


def kernel(q, k, v, cu_seqlens, max_seqlen=None, **_unused):
    global LAST_RESULT
    from concourse.bass_utils import run_bass_kernel_spmd

    q = np.ascontiguousarray(np.asarray(q, dtype=np.float32))
    k = np.ascontiguousarray(np.asarray(k, dtype=np.float32))
    v = np.ascontiguousarray(np.asarray(v, dtype=np.float32))
    cu = np.asarray(cu_seqlens).astype(np.int64)
    lens = tuple(int(cu[i + 1] - cu[i]) for i in range(len(cu) - 1))
    T = int(cu[-1])
    assert q.shape == (T, NUM_HEADS * HEAD_DIM)

    nc = _get_program(lens)

    in_maps = []
    for c in range(N_CORES):
        in_maps.append(
            {
                "q": np.ascontiguousarray(q[:, c * DQ : (c + 1) * DQ]),
                "k": np.ascontiguousarray(
                    k[:, c * HEAD_DIM : (c + 1) * HEAD_DIM]
                ),
                "v": np.ascontiguousarray(
                    v[:, c * HEAD_DIM : (c + 1) * HEAD_DIM]
                ),
            }
        )

    trace = bool(int(os.environ.get("KERNEL_TRACE", "0")))
    LAST_RESULT = run_bass_kernel_spmd(
        nc, in_maps, core_ids=list(range(N_CORES)), trace=trace
    )
    out = np.concatenate(
        [LAST_RESULT.results[c]["out"] for c in range(N_CORES)], axis=1
    )
    return out.reshape(T, NUM_HEADS, HEAD_DIM).astype(np.float32)
